# revision 16
# baseline (speedup 1.0000x reference)
"""Trainium2 Bass kernel for nn_Discriminator_11012296147416.

2-layer GCN (100K nodes, 1.6M edges + self-loops) -> node-product aggregation
-> tiny MLP head. Returns (out [1,1], g [1,64]) matching the reference.

Strategy (8 NeuronCores, SPMD single program):
  * Edges are sorted by target node (col) on the host; each core owns a
    contiguous range of 98 128-node output tiles (784 tiles = 100352 padded
    nodes). No cross-core psum needed for the scatter - each core produces a
    disjoint slice of the conv output.
  * Per 128-col tile, edges are packed into chunks of 128 (C_MAX chunks per
    tile, uniform across cores so the SPMD program is identical). The gather
    table rows h[row]*dinv[row] are fetched with one batched indirect DMA per
    GT tiles; scatter-add is a one-hot (iota==col_local) matmul accumulated in
    PSUM. Degree normalization dinv[col] is applied by the ACT epilogue
    (per-partition scale); the bias is pre-added inside PSUM as a rank-1
    outer(sqrt(deg), b) so tanh is a single fused ACT op.
  * Layer-2 features (12544 rows per core) are AllGathered (bf16) so every
    core can gather any node's features.
  * Node-validity-masked running product over owned nodes, partition-product
    via PE transpose + fold, AllGather of 8 partials, replicated MLP head.
"""

import sys
import types
import numpy as np
import ml_dtypes

import concourse.bass as bass
import concourse.mybir as mybir
import concourse.tile as tile
from concourse.bass_utils import run_bass_kernel_spmd
from concourse.masks import make_identity

P = 128
BF16 = ml_dtypes.bfloat16

_LAST_CMAX = None

FULL_CFG = dict(
    n_cores=8,
    tpc=98,          # 128-node tiles per core (8*98*128 = 100352 >= 100000)
    n_nodes=100000,
    m_dim=28,
    f1=128,
    f2=64,
    gt=2,            # tiles per batched gather instruction
)


def _register_profile_hook():
    """The image's antenv lacks axon_hooks; register the ctypes NTFF hook so
    run_bass_kernel_spmd(trace=True) can try to profile."""
    if "antenv.axon_hooks" in sys.modules:
        return
    try:
        from trn_agent_boot import trn_boot
        real = trn_boot._ntff_profile_via_ctypes("/opt/axon/libaxon_pjrt.so")

        def hook(output_dir, device_ids):
            # device id 0 maps to nothing on this terminal (NC_v30-37);
            # profile all devices instead.
            return real(output_dir, None)

        m = types.ModuleType("antenv.axon_hooks")
        m.get_axon_ntff_profile_hook = lambda: hook
        sys.modules["antenv.axon_hooks"] = m
    except Exception:
        pass


def prep(x, edge_index, cfg):
    """Host-side sharding/layout of the integer edge structure.

    Returns (meta, shared_arrays, per_core_arrays).
    """
    n_cores = cfg["n_cores"]
    tpc = cfg["tpc"]
    N = cfg["n_nodes"]
    TT = n_cores * tpc
    NPAD = TT * P
    assert NPAD >= N

    ei = np.asarray(edge_index)
    rows = np.concatenate([ei[0], np.arange(N, dtype=np.int64)])
    cols = np.concatenate([ei[1], np.arange(N, dtype=np.int64)])

    deg = np.bincount(cols, minlength=NPAD).astype(np.float32)
    deg[N:] = 1.0  # ghost nodes: harmless nonzero degree

    order = np.argsort(cols, kind="stable")
    rows_s = rows[order].astype(np.int64)
    cols_s = cols[order].astype(np.int64)
    tile_id = cols_s >> 7

    tcnt = np.bincount(tile_id, minlength=TT)
    cmax = int(np.ceil(tcnt.max() / P))

    ridx = np.zeros((P, TT * cmax), np.int32)          # pad -> row 0 (harmless read)
    cloc = np.full((P, TT * cmax), -1.0, np.float32)   # pad -> no one-hot match
    tstart = np.zeros(TT + 1, np.int64)
    tstart[1:] = np.cumsum(tcnt)
    j_in_tile = np.arange(len(cols_s), dtype=np.int64) - tstart[tile_id]
    cc = j_in_tile >> 7
    pp = j_in_tile & 127
    colidx = tile_id * cmax + cc
    ridx[pp, colidx] = rows_s.astype(np.int32)
    cloc[pp, colidx] = (cols_s & 127).astype(np.float32)

    node_ids = np.arange(NPAD, dtype=np.int64)
    valid = (node_ids < N).astype(np.float32)
    deg_tiled = deg.reshape(TT, P).T.copy()            # [P, TT]
    msk_tiled = valid.reshape(TT, P).T.copy()          # [P, TT]

    xT = np.zeros((cfg["m_dim"], NPAD), np.float32)
    xT[:, :N] = np.asarray(x, np.float32).T

    meta = dict(cmax=cmax, TT=TT, NPAD=NPAD)
    shared = dict(xT=xT, dega=deg_tiled)
    per_core = []
    for k in range(n_cores):
        sl = slice(k * tpc * cmax, (k + 1) * tpc * cmax)
        to = slice(k * tpc, (k + 1) * tpc)
        per_core.append(dict(
            ridx=np.ascontiguousarray(ridx[:, sl]),
            cloc=np.ascontiguousarray(cloc[:, sl]),
            dego=np.ascontiguousarray(deg_tiled[:, to]),
            msk=np.ascontiguousarray(msk_tiled[:, to]),
            onem=np.ascontiguousarray(1.0 - msk_tiled[:, to]),
        ))
    return meta, shared, per_core


def split_excess_waits(nc, limit=4):
    """Walrus caps per-instruction sync-wait commands. Hoist excess waits into
    preceding same-engine NoOps (sequencer blocks on each in order, so the
    semantics are identical)."""
    fn = nc.m.functions[0]
    ctr = 0
    for bb in fn.blocks:
        changed = False
        new = []
        for ins in bb.instructions:
            si = ins.sync_info
            waits = list(si.on_wait) if (si and si.on_wait) else []
            if len(waits) > limit:
                changed = True
                excess, keep = waits[:-limit], waits[-limit:]
                for i in range(0, len(excess), limit):
                    chunk = excess[i : i + limit]
                    nop = mybir.InstNoOp(name=f"WSPLIT-{ctr}", ins=[], outs=[])
                    ctr += 1
                    nop.engine = ins.engine
                    nop.sync_info = mybir.SyncInfo(on_wait=chunk, on_update=[])
                    new.append(nop)
                ins.sync_info = mybir.SyncInfo(
                    on_wait=keep, on_update=list(si.on_update or []))
            new.append(ins)
        if changed:
            try:
                bb.instructions[:] = new
            except TypeError:
                bb.instructions = new
    return ctr


def build_program(cfg, cmax, collectives=True, split_waits=True):
    n_cores = cfg["n_cores"]
    tpc = cfg["tpc"]
    M = cfg["m_dim"]
    F1 = cfg["f1"]
    F2 = cfg["f2"]
    GT = cfg["gt"]
    TT = n_cores * tpc
    NPAD = TT * P
    assert tpc % GT == 0

    dt = mybir.dt
    Act = mybir.ActivationFunctionType
    Alu = mybir.AluOpType

    nc = bass.Bass(num_devices=n_cores if collectives else 1, debug=False)

    # ---- external inputs ----
    xT_d = nc.dram_tensor("xT", [M, NPAD], dt.float32, kind="ExternalInput")
    ridx_d = nc.dram_tensor("ridx", [P, tpc * cmax], dt.int32, kind="ExternalInput")
    cloc_d = nc.dram_tensor("cloc", [P, tpc * cmax], dt.float32, kind="ExternalInput")
    dega_d = nc.dram_tensor("dega", [P, TT], dt.float32, kind="ExternalInput")
    dego_d = nc.dram_tensor("dego", [P, tpc], dt.float32, kind="ExternalInput")
    msk_d = nc.dram_tensor("msk", [P, tpc], dt.float32, kind="ExternalInput")
    onem_d = nc.dram_tensor("onem", [P, tpc], dt.float32, kind="ExternalInput")
    w1_d = nc.dram_tensor("w1", [M, F1], dt.float32, kind="ExternalInput")
    b1_d = nc.dram_tensor("b1", [1, F1], dt.float32, kind="ExternalInput")
    w2_d = nc.dram_tensor("w2", [F1, F2], dt.float32, kind="ExternalInput")
    b2_d = nc.dram_tensor("b2", [1, F2], dt.float32, kind="ExternalInput")
    wd1_d = nc.dram_tensor("wd1", [F2, 128], dt.float32, kind="ExternalInput")
    bd1_d = nc.dram_tensor("bd1", [128, 1], dt.float32, kind="ExternalInput")
    wd2_d = nc.dram_tensor("wd2", [128, F2], dt.float32, kind="ExternalInput")
    bd2_d = nc.dram_tensor("bd2", [F2, 1], dt.float32, kind="ExternalInput")
    wo_d = nc.dram_tensor("wo", [F2, 1], dt.float32, kind="ExternalInput")
    bo_d = nc.dram_tensor("bo", [1, 1], dt.float32, kind="ExternalInput")

    # ---- external outputs ----
    out_d = nc.dram_tensor("out", [1, 1], dt.float32, kind="ExternalOutput")
    g_d = nc.dram_tensor("g", [1, F2], dt.float32, kind="ExternalOutput")

    # ---- internal dram ----
    hs1_d = nc.dram_tensor("hs1", [NPAD, F1], dt.bfloat16)
    hs2s_d = nc.dram_tensor("hs2s", [tpc * P, F2], dt.bfloat16)
    hs2_d = nc.dram_tensor("hs2", [NPAD, F2], dt.bfloat16, addr_space="Shared")
    sq_scr = nc.dram_tensor("sq_scr", [1, tpc * P], dt.float32)
    ag_in = nc.dram_tensor("ag_in", [1, F2], dt.float32)
    ag_out = nc.dram_tensor("ag_out", [n_cores, F2], dt.float32, addr_space="Shared")
    ptr_d = nc.dram_tensor("ptr", [F2, 1], dt.float32)

    groups = [list(range(n_cores))]

    with tile.TileContext(nc) as tc:
        with (
            tc.tile_pool(name="const", bufs=1) as cp,
            tc.tile_pool(name="cpsum", bufs=1, space="PSUM") as cpp,
        ):
            # constants
            iota_i = cp.tile([P, P], dt.int32)
            nc.gpsimd.iota(iota_i[:], pattern=[[1, P]], base=0, channel_multiplier=0)
            iota_b = cp.tile([P, P], dt.bfloat16)
            nc.vector.tensor_copy(iota_b[:], iota_i[:])
            ident_b = cp.tile([P, P], dt.bfloat16)
            make_identity(nc, ident_b[:])
            ident_f = cp.tile([P, P], dt.float32)
            make_identity(nc, ident_f[:])

            idx_t = cp.tile([P, tpc * cmax], dt.int32)
            nc.sync.dma_start(idx_t[:], ridx_d[:, :])
            cl_t = cp.tile([P, tpc * cmax], dt.float32)
            nc.sync.dma_start(cl_t[:], cloc_d[:, :])

            dega_t = cp.tile([P, TT], dt.float32)
            nc.sync.dma_start(dega_t[:], dega_d[:, :])
            sqa_t = cp.tile([P, TT], dt.float32)
            nc.scalar.activation(sqa_t[:], dega_t[:], Act.Sqrt)
            dinva_t = cp.tile([P, TT], dt.float32)
            nc.vector.reciprocal(dinva_t[:], sqa_t[:])

            dego_t = cp.tile([P, tpc], dt.float32)
            nc.sync.dma_start(dego_t[:], dego_d[:, :])
            sqo_t = cp.tile([P, tpc], dt.float32)
            nc.scalar.activation(sqo_t[:], dego_t[:], Act.Sqrt)
            dinvo_t = cp.tile([P, tpc], dt.float32)
            nc.vector.reciprocal(dinvo_t[:], sqo_t[:])

            # sqrt(deg) rows [1, tpc*128] via PE transpose + DRAM roundtrip
            psq = cpp.tile([tpc, P], dt.float32)
            nc.tensor.transpose(psq[:], sqo_t[:], ident_f[:])
            sqoT = cp.tile([tpc, P], dt.float32)
            nc.vector.tensor_copy(sqoT[:], psq[:])
            nc.sync.dma_start(sq_scr[:, :], sqoT[:])
            sqrow_t = cp.tile([1, tpc * P], dt.float32)
            nc.sync.dma_start(sqrow_t[:], sq_scr[:, :])

            msk_t = cp.tile([P, tpc], dt.float32)
            nc.sync.dma_start(msk_t[:], msk_d[:, :])
            onem_t = cp.tile([P, tpc], dt.float32)
            nc.sync.dma_start(onem_t[:], onem_d[:, :])

            w1_t = cp.tile([M, F1], dt.float32)
            nc.sync.dma_start(w1_t[:], w1_d[:, :])
            b1_t = cp.tile([1, F1], dt.float32)
            nc.sync.dma_start(b1_t[:], b1_d[:, :])
            w2f_t = cp.tile([F1, F2], dt.float32)
            nc.sync.dma_start(w2f_t[:], w2_d[:, :])
            w2_t = cp.tile([F1, F2], dt.bfloat16)
            nc.vector.tensor_copy(w2_t[:], w2f_t[:])
            b2_t = cp.tile([1, F2], dt.float32)
            nc.sync.dma_start(b2_t[:], b2_d[:, :])

            acc = cp.tile([P, F2], dt.float32)
            nc.vector.memset(acc[:], 1.0)

            # ---- phase 1: hs1 = (x @ W1) * dinv[node], node-major bf16 ----
            BG = 4  # node-blocks per psum tile
            with (
                tc.tile_pool(name="p1sb", bufs=4) as sp1,
                tc.tile_pool(name="p1ps", bufs=3, space="PSUM") as pp1,
            ):
                for g0 in range(0, TT, BG):
                    nb = min(BG, TT - g0)
                    xc = sp1.tile([M, BG * P], dt.float32, tag="xc")
                    nc.sync.dma_start(xc[:, : nb * P], xT_d[:, g0 * P : (g0 + nb) * P])
                    ps = pp1.tile([P, BG * F1], dt.float32, tag="ps0")
                    hsb = sp1.tile([P, BG * F1], dt.bfloat16, tag="hsb")
                    for j in range(nb):
                        b = g0 + j
                        nc.tensor.matmul(
                            ps[:, j * F1 : (j + 1) * F1],
                            xc[:, j * P : (j + 1) * P],
                            w1_t[:],
                            start=True, stop=True,
                        )
                        nc.scalar.activation(
                            hsb[:, j * F1 : (j + 1) * F1],
                            ps[:, j * F1 : (j + 1) * F1],
                            Act.Copy, scale=dinva_t[:, b : b + 1],
                        )
                    nc.sync.dma_start(
                        hs1_d.ap().rearrange("(t p) f -> p t f", p=P)[:, g0 : g0 + nb, :],
                        hsb[:, : nb * F1],
                    )

            tc.no_sync_barrier()

            # ---- phase 2: layer-1 conv + project to layer-2 table ----
            with (
                tc.tile_pool(name="p2sb", bufs=3) as sp2,
                tc.tile_pool(name="p2eq", bufs=8) as sp2e,
                tc.tile_pool(name="p2ps", bufs=2, space="PSUM") as pp2,
            ):
                for g in range(tpc // GT):
                    gt1 = sp2.tile([P, GT * cmax * F1], dt.bfloat16, tag="g1")
                    nc.gpsimd.indirect_dma_start(
                        out=gt1[:], out_offset=None,
                        in_=hs1_d[:, :],
                        in_offset=bass.IndirectOffsetOnAxis(
                            ap=idx_t[:, g * GT * cmax : (g + 1) * GT * cmax], axis=0),
                    )
                    for tl in range(GT):
                        t = g * GT + tl
                        ps1 = pp2.tile([P, F1], dt.float32, tag="ps1")
                        for c in range(cmax):
                            col = t * cmax + c
                            eq = sp2e.tile([P, P], dt.bfloat16, tag="eq")
                            nc.vector.tensor_scalar(
                                eq[:], iota_b[:], cl_t[:, col : col + 1], None,
                                Alu.is_equal)
                            nc.tensor.matmul(
                                ps1[:], eq[:],
                                gt1[:, (tl * cmax + c) * F1 : (tl * cmax + c + 1) * F1],
                                start=(c == 0), stop=False)
                        nc.tensor.matmul(
                            ps1[:], sqrow_t[:, t * P : (t + 1) * P], b1_t[:],
                            start=False, stop=True)
                        h1 = sp2.tile([P, F1], dt.bfloat16, tag="h1")
                        nc.scalar.activation(
                            h1[:], ps1[:], Act.Tanh, scale=dinvo_t[:, t : t + 1])
                        t2 = sp2.tile([P, F1], dt.bfloat16, tag="t2")
                        nc.scalar.activation(
                            t2[:], h1[:], Act.Copy, scale=dinvo_t[:, t : t + 1])
                        psT = pp2.tile([P, F1], dt.bfloat16, tag="psT")
                        nc.tensor.transpose(psT[:], t2[:], ident_b[:])
                        t2T = sp2.tile([P, F1], dt.bfloat16, tag="t2T")
                        nc.vector.tensor_copy(t2T[:], psT[:])
                        ps2 = pp2.tile([P, F2], dt.float32, tag="ps2")
                        nc.tensor.matmul(ps2[:], t2T[:], w2_t[:], start=True, stop=True)
                        hs2sb = sp2.tile([P, F2], dt.bfloat16, tag="hs2sb")
                        nc.scalar.activation(hs2sb[:], ps2[:], Act.Copy)
                        nc.sync.dma_start(hs2s_d[t * P : (t + 1) * P, :], hs2sb[:])

            tc.no_sync_barrier()

            # ---- phase 3: allgather layer-2 table ----
            if collectives:
                nc.gpsimd.collective_compute(
                    "AllGather", Alu.bypass, replica_groups=groups,
                    ins=[hs2s_d[:, :]], outs=[hs2_d[:, :]])
            else:
                nc.sync.dma_start(hs2_d[0 : tpc * P, :], hs2s_d[:, :])

            tc.no_sync_barrier()

            # ---- phase 4: layer-2 conv + masked product ----
            with (
                tc.tile_pool(name="p4sb", bufs=3) as sp4,
                tc.tile_pool(name="p4eq", bufs=8) as sp4e,
                tc.tile_pool(name="p4ps", bufs=2, space="PSUM") as pp4,
            ):
                for g in range(tpc // GT):
                    gt2 = sp4.tile([P, GT * cmax * F2], dt.bfloat16, tag="g2")
                    nc.gpsimd.indirect_dma_start(
                        out=gt2[:], out_offset=None,
                        in_=hs2_d[:, :],
                        in_offset=bass.IndirectOffsetOnAxis(
                            ap=idx_t[:, g * GT * cmax : (g + 1) * GT * cmax], axis=0),
                    )
                    for tl in range(GT):
                        t = g * GT + tl
                        ps1b = pp4.tile([P, F2], dt.float32, tag="ps1b")
                        for c in range(cmax):
                            col = t * cmax + c
                            eq = sp4e.tile([P, P], dt.bfloat16, tag="eq4")
                            nc.vector.tensor_scalar(
                                eq[:], iota_b[:], cl_t[:, col : col + 1], None,
                                Alu.is_equal)
                            nc.tensor.matmul(
                                ps1b[:], eq[:],
                                gt2[:, (tl * cmax + c) * F2 : (tl * cmax + c + 1) * F2],
                                start=(c == 0), stop=False)
                        nc.tensor.matmul(
                            ps1b[:], sqrow_t[:, t * P : (t + 1) * P], b2_t[:],
                            start=False, stop=True)
                        h2 = sp4.tile([P, F2], dt.float32, tag="h2")
                        nc.scalar.activation(
                            h2[:], ps1b[:], Act.Tanh, scale=dinvo_t[:, t : t + 1])
                        h2m = sp4.tile([P, F2], dt.float32, tag="h2m")
                        nc.vector.tensor_scalar(
                            h2m[:], h2[:], msk_t[:, t : t + 1], onem_t[:, t : t + 1],
                            Alu.mult, Alu.add)
                        nc.vector.tensor_mul(acc[:], acc[:], h2m[:])

            tc.no_sync_barrier()

            # ---- phase 5: partition product, cross-core product, MLP head ----
            with (
                tc.tile_pool(name="p5sb", bufs=1) as sp5,
                tc.tile_pool(name="p5ps", bufs=1, space="PSUM") as pp5,
            ):
                psa = pp5.tile([F2, P], dt.float32)
                nc.tensor.transpose(psa[:], acc[:], ident_f[:])
                accT = sp5.tile([F2, P], dt.float32)
                nc.vector.tensor_copy(accT[:], psa[:])
                w = P
                while w > 1:
                    w //= 2
                    nc.vector.tensor_mul(accT[:, :w], accT[:, :w], accT[:, w : 2 * w])
                nc.sync.dma_start(ag_in[:, :], accT[:, :1])
                if collectives:
                    nc.gpsimd.collective_compute(
                        "AllGather", Alu.bypass, replica_groups=groups,
                        ins=[ag_in[:, :]], outs=[ag_out[:, :]])
                else:
                    nc.sync.dma_start(ag_out[0:1, :], ag_in[:, :])
                all_t = sp5.tile([1, n_cores * F2], dt.float32)
                nc.sync.dma_start(all_t[:], ag_out[:, :])
                w = n_cores * F2
                while w > F2:
                    w //= 2
                    nc.vector.tensor_mul(all_t[:, :w], all_t[:, :w], all_t[:, w : 2 * w])
                nc.sync.dma_start(ptr_d[:, :], all_t[:, :F2])
                pT = sp5.tile([F2, 1], dt.float32)
                nc.sync.dma_start(pT[:], ptr_d[:, :])

                wd1_t = sp5.tile([F2, 128], dt.float32)
                nc.sync.dma_start(wd1_t[:], wd1_d[:, :])
                bd1_t = sp5.tile([128, 1], dt.float32)
                nc.sync.dma_start(bd1_t[:], bd1_d[:, :])
                ps1m = pp5.tile([128, 1], dt.float32)
                nc.tensor.matmul(ps1m[:], wd1_t[:], pT[:], start=True, stop=True)
                g1_t = sp5.tile([128, 1], dt.float32)
                nc.scalar.activation(g1_t[:], ps1m[:], Act.Tanh, bias=bd1_t[:, :1])

                wd2_t = sp5.tile([128, F2], dt.float32)
                nc.sync.dma_start(wd2_t[:], wd2_d[:, :])
                bd2_t = sp5.tile([F2, 1], dt.float32)
                nc.sync.dma_start(bd2_t[:], bd2_d[:, :])
                ps2m = pp5.tile([F2, 1], dt.float32)
                nc.tensor.matmul(ps2m[:], wd2_t[:], g1_t[:], start=True, stop=True)
                g2_t = sp5.tile([F2, 1], dt.float32)
                nc.scalar.activation(g2_t[:], ps2m[:], Act.Tanh, bias=bd2_t[:, :1])
                nc.sync.dma_start(g_d[0, :, None], g2_t[:])

                wo_t = sp5.tile([F2, 1], dt.float32)
                nc.sync.dma_start(wo_t[:], wo_d[:, :])
                pso = pp5.tile([1, 1], dt.float32)
                nc.tensor.matmul(pso[:], wo_t[:], g2_t[:], start=True, stop=True)
                bo_t = sp5.tile([1, 1], dt.float32)
                nc.sync.dma_start(bo_t[:], bo_d[:, :])
                o_t = sp5.tile([1, 1], dt.float32)
                nc.vector.tensor_add(o_t[:], pso[:], bo_t[:])
                nc.sync.dma_start(out_d[:, :], o_t[:])

    if split_waits:
        split_excess_waits(nc, limit=1)
    return nc


def make_in_maps(cfg, inputs, meta, shared, per_core):
    w1 = np.asarray(inputs["W1"], np.float32)
    b1 = np.asarray(inputs["b1"], np.float32)
    b2 = np.asarray(inputs["b2"], np.float32)
    common = dict(
        xT=shared["xT"], dega=shared["dega"],
        w1=w1, b1=b1[None, :], w2=np.asarray(inputs["W2"], np.float32), b2=b2[None, :],
        wd1=np.asarray(inputs["Wd1"], np.float32),
        bd1=np.asarray(inputs["bd1"], np.float32).reshape(-1, 1),
        wd2=np.asarray(inputs["Wd2"], np.float32),
        bd2=np.asarray(inputs["bd2"], np.float32).reshape(-1, 1),
        wo=np.asarray(inputs["Wo"], np.float32),
        bo=np.asarray(inputs["bo"], np.float32).reshape(1, 1),
    )
    return [dict(common, **pc) for pc in per_core]


def run(inputs, cfg=None, trace=False):
    cfg = cfg or FULL_CFG
    _register_profile_hook()
    meta, shared, per_core = prep(inputs["x"], inputs["edge_index"], cfg)
    global _LAST_CMAX
    _LAST_CMAX = meta["cmax"]
    nc = build_program(cfg, meta["cmax"])
    in_maps = make_in_maps(cfg, inputs, meta, shared, per_core)
    res = run_bass_kernel_spmd(
        nc, in_maps, list(range(cfg["n_cores"])), trace=trace)
    r0 = res.results[0]
    return (np.asarray(r0["out"], np.float32),
            np.asarray(r0["g"], np.float32)), res


def kernel(**inputs):
    (out, g), _ = run(inputs)
    return (out, g)


# revision 21
# speedup vs baseline: 1.0181x; 1.0181x over previous
"""Trainium2 Bass kernel for nn_Discriminator_11012296147416.

2-layer GCN (100K nodes, 1.6M edges + self-loops) -> node-product aggregation
-> tiny MLP head. Returns (out [1,1], g [1,64]) matching the reference.

Strategy (8 NeuronCores, SPMD single program):
  * Edges are sorted by target node (col) on the host; each core owns a
    contiguous range of 98 128-node output tiles (784 tiles = 100352 padded
    nodes). No cross-core psum needed for the scatter - each core produces a
    disjoint slice of the conv output.
  * Per 128-col tile, edges are packed into chunks of 128 (C_MAX chunks per
    tile, uniform across cores so the SPMD program is identical). The gather
    table rows h[row]*dinv[row] are fetched with one batched indirect DMA per
    GT tiles; scatter-add is a one-hot (iota==col_local) matmul accumulated in
    PSUM. Degree normalization dinv[col] is applied by the ACT epilogue
    (per-partition scale); the bias is pre-added inside PSUM as a rank-1
    outer(sqrt(deg), b) so tanh is a single fused ACT op.
  * Layer-2 features (12544 rows per core) are AllGathered (bf16) so every
    core can gather any node's features.
  * Node-validity-masked running product over owned nodes, partition-product
    via PE transpose + fold, AllGather of 8 partials, replicated MLP head.
"""

import sys
import types
import numpy as np
import ml_dtypes

import concourse.bass as bass
import concourse.mybir as mybir
import concourse.tile as tile
from concourse.bass_utils import run_bass_kernel_spmd
from concourse.masks import make_identity

P = 128
BF16 = ml_dtypes.bfloat16

_LAST_CMAX = None

FULL_CFG = dict(
    n_cores=8,
    tpc=98,          # 128-node tiles per core (8*98*128 = 100352 >= 100000)
    n_nodes=100000,
    m_dim=28,
    f1=128,
    f2=64,
    gt=2,            # tiles per batched gather instruction
)


def _register_profile_hook():
    """The image's antenv lacks axon_hooks; register the ctypes NTFF hook so
    run_bass_kernel_spmd(trace=True) can try to profile."""
    if "antenv.axon_hooks" in sys.modules:
        return
    try:
        from trn_agent_boot import trn_boot
        real = trn_boot._ntff_profile_via_ctypes("/opt/axon/libaxon_pjrt.so")

        def hook(output_dir, device_ids):
            # device id 0 maps to nothing on this terminal (NC_v30-37);
            # profile all devices instead.
            return real(output_dir, None)

        m = types.ModuleType("antenv.axon_hooks")
        m.get_axon_ntff_profile_hook = lambda: hook
        sys.modules["antenv.axon_hooks"] = m
    except Exception:
        pass


def prep(x, edge_index, cfg):
    """Host-side sharding/layout of the integer edge structure.

    Returns (meta, shared_arrays, per_core_arrays).
    """
    n_cores = cfg["n_cores"]
    tpc = cfg["tpc"]
    N = cfg["n_nodes"]
    TT = n_cores * tpc
    NPAD = TT * P
    assert NPAD >= N

    ei = np.asarray(edge_index)
    rows = np.concatenate([ei[0], np.arange(N, dtype=np.int64)])
    cols = np.concatenate([ei[1], np.arange(N, dtype=np.int64)])

    deg = np.bincount(cols, minlength=NPAD).astype(np.float32)
    deg[N:] = 1.0  # ghost nodes: harmless nonzero degree

    order = np.argsort(cols, kind="stable")
    rows_s = rows[order].astype(np.int64)
    cols_s = cols[order].astype(np.int64)
    tile_id = cols_s >> 7

    tcnt = np.bincount(tile_id, minlength=TT)
    cmax = int(np.ceil(tcnt.max() / P))

    ridx = np.zeros((P, TT * cmax), np.int32)          # pad -> row 0 (harmless read)
    cloc = np.full((P, TT * cmax), -1.0, np.float32)   # pad -> no one-hot match
    tstart = np.zeros(TT + 1, np.int64)
    tstart[1:] = np.cumsum(tcnt)
    j_in_tile = np.arange(len(cols_s), dtype=np.int64) - tstart[tile_id]
    cc = j_in_tile >> 7
    pp = j_in_tile & 127
    colidx = tile_id * cmax + cc
    ridx[pp, colidx] = rows_s.astype(np.int32)
    cloc[pp, colidx] = (cols_s & 127).astype(np.float32)

    node_ids = np.arange(NPAD, dtype=np.int64)
    valid = (node_ids < N).astype(np.float32)
    deg_tiled = deg.reshape(TT, P).T.copy()            # [P, TT]
    msk_tiled = valid.reshape(TT, P).T.copy()          # [P, TT]

    xT = np.zeros((cfg["m_dim"], NPAD), np.float32)
    xT[:, :N] = np.asarray(x, np.float32).T

    meta = dict(cmax=cmax, TT=TT, NPAD=NPAD)
    shared = dict(xT=xT, dega=deg_tiled)
    per_core = []
    for k in range(n_cores):
        sl = slice(k * tpc * cmax, (k + 1) * tpc * cmax)
        to = slice(k * tpc, (k + 1) * tpc)
        per_core.append(dict(
            ridx=np.ascontiguousarray(ridx[:, sl]),
            cloc=np.ascontiguousarray(cloc[:, sl]),
            dego=np.ascontiguousarray(deg_tiled[:, to]),
            msk=np.ascontiguousarray(msk_tiled[:, to]),
            onem=np.ascontiguousarray(1.0 - msk_tiled[:, to]),
        ))
    return meta, shared, per_core


def split_excess_waits(nc, limit=4):
    """Walrus caps per-instruction sync-wait commands. Hoist excess waits into
    preceding same-engine NoOps (sequencer blocks on each in order, so the
    semantics are identical)."""
    fn = nc.m.functions[0]
    ctr = 0
    for bb in fn.blocks:
        changed = False
        new = []
        for ins in bb.instructions:
            si = ins.sync_info
            waits = list(si.on_wait) if (si and si.on_wait) else []
            if len(waits) > limit:
                changed = True
                excess, keep = waits[:-limit], waits[-limit:]
                for i in range(0, len(excess), limit):
                    chunk = excess[i : i + limit]
                    nop = mybir.InstNoOp(name=f"WSPLIT-{ctr}", ins=[], outs=[])
                    ctr += 1
                    nop.engine = ins.engine
                    nop.sync_info = mybir.SyncInfo(on_wait=chunk, on_update=[])
                    new.append(nop)
                ins.sync_info = mybir.SyncInfo(
                    on_wait=keep, on_update=list(si.on_update or []))
            new.append(ins)
        if changed:
            try:
                bb.instructions[:] = new
            except TypeError:
                bb.instructions = new
    return ctr


def build_program(cfg, cmax, collectives=True, split_waits=True):
    n_cores = cfg["n_cores"]
    tpc = cfg["tpc"]
    M = cfg["m_dim"]
    F1 = cfg["f1"]
    F2 = cfg["f2"]
    GT = cfg["gt"]
    TT = n_cores * tpc
    NPAD = TT * P
    assert tpc % GT == 0

    dt = mybir.dt
    Act = mybir.ActivationFunctionType
    Alu = mybir.AluOpType

    nc = bass.Bass(num_devices=n_cores if collectives else 1, debug=False)

    # ---- external inputs ----
    xT_d = nc.dram_tensor("xT", [M, NPAD], dt.float32, kind="ExternalInput")
    ridx_d = nc.dram_tensor("ridx", [P, tpc * cmax], dt.int32, kind="ExternalInput")
    cloc_d = nc.dram_tensor("cloc", [P, tpc * cmax], dt.float32, kind="ExternalInput")
    dega_d = nc.dram_tensor("dega", [P, TT], dt.float32, kind="ExternalInput")
    dego_d = nc.dram_tensor("dego", [P, tpc], dt.float32, kind="ExternalInput")
    msk_d = nc.dram_tensor("msk", [P, tpc], dt.float32, kind="ExternalInput")
    onem_d = nc.dram_tensor("onem", [P, tpc], dt.float32, kind="ExternalInput")
    w1_d = nc.dram_tensor("w1", [M, F1], dt.float32, kind="ExternalInput")
    b1_d = nc.dram_tensor("b1", [1, F1], dt.float32, kind="ExternalInput")
    w2_d = nc.dram_tensor("w2", [F1, F2], dt.float32, kind="ExternalInput")
    b2_d = nc.dram_tensor("b2", [1, F2], dt.float32, kind="ExternalInput")
    wd1_d = nc.dram_tensor("wd1", [F2, 128], dt.float32, kind="ExternalInput")
    bd1_d = nc.dram_tensor("bd1", [128, 1], dt.float32, kind="ExternalInput")
    wd2_d = nc.dram_tensor("wd2", [128, F2], dt.float32, kind="ExternalInput")
    bd2_d = nc.dram_tensor("bd2", [F2, 1], dt.float32, kind="ExternalInput")
    wo_d = nc.dram_tensor("wo", [F2, 1], dt.float32, kind="ExternalInput")
    bo_d = nc.dram_tensor("bo", [1, 1], dt.float32, kind="ExternalInput")

    # ---- external outputs ----
    out_d = nc.dram_tensor("out", [1, 1], dt.float32, kind="ExternalOutput")
    g_d = nc.dram_tensor("g", [1, F2], dt.float32, kind="ExternalOutput")

    # ---- internal dram ----
    hs1_d = nc.dram_tensor("hs1", [NPAD, F1], dt.bfloat16)
    hs2s_d = nc.dram_tensor("hs2s", [tpc * P, F2], dt.bfloat16)
    hs2_d = nc.dram_tensor("hs2", [NPAD, F2], dt.bfloat16, addr_space="Shared")
    sq_scr = nc.dram_tensor("sq_scr", [1, tpc * P], dt.float32)
    ag_in = nc.dram_tensor("ag_in", [1, F2], dt.float32)
    ag_out = nc.dram_tensor("ag_out", [n_cores, F2], dt.float32, addr_space="Shared")
    ptr_d = nc.dram_tensor("ptr", [F2, 1], dt.float32)

    groups = [list(range(n_cores))]

    with tile.TileContext(nc) as tc:
        with (
            tc.tile_pool(name="const", bufs=1) as cp,
            tc.tile_pool(name="cpsum", bufs=1, space="PSUM") as cpp,
        ):
            # constants
            iota_i = cp.tile([P, P], dt.int32)
            nc.gpsimd.iota(iota_i[:], pattern=[[1, P]], base=0, channel_multiplier=0)
            iota_b = cp.tile([P, P], dt.bfloat16)
            nc.vector.tensor_copy(iota_b[:], iota_i[:])
            ident_b = cp.tile([P, P], dt.bfloat16)
            make_identity(nc, ident_b[:])
            ident_f = cp.tile([P, P], dt.float32)
            make_identity(nc, ident_f[:])

            idx_t = cp.tile([P, tpc * cmax], dt.int32)
            nc.sync.dma_start(idx_t[:], ridx_d[:, :])
            cl_t = cp.tile([P, tpc * cmax], dt.float32)
            nc.sync.dma_start(cl_t[:], cloc_d[:, :])

            dega_t = cp.tile([P, TT], dt.float32)
            nc.sync.dma_start(dega_t[:], dega_d[:, :])
            sqa_t = cp.tile([P, TT], dt.float32)
            nc.scalar.activation(sqa_t[:], dega_t[:], Act.Sqrt)
            dinva_t = cp.tile([P, TT], dt.float32)
            nc.vector.reciprocal(dinva_t[:], sqa_t[:])

            dego_t = cp.tile([P, tpc], dt.float32)
            nc.sync.dma_start(dego_t[:], dego_d[:, :])
            sqo_t = cp.tile([P, tpc], dt.float32)
            nc.scalar.activation(sqo_t[:], dego_t[:], Act.Sqrt)
            dinvo_t = cp.tile([P, tpc], dt.float32)
            nc.vector.reciprocal(dinvo_t[:], sqo_t[:])

            # sqrt(deg) rows [1, tpc*128] via PE transpose + DRAM roundtrip
            psq = cpp.tile([tpc, P], dt.float32)
            nc.tensor.transpose(psq[:], sqo_t[:], ident_f[:])
            sqoT = cp.tile([tpc, P], dt.float32)
            nc.vector.tensor_copy(sqoT[:], psq[:])
            nc.sync.dma_start(sq_scr[:, :], sqoT[:])
            sqrow_t = cp.tile([1, tpc * P], dt.float32)
            nc.sync.dma_start(sqrow_t[:], sq_scr[:, :])

            msk_t = cp.tile([P, tpc], dt.float32)
            nc.sync.dma_start(msk_t[:], msk_d[:, :])
            onem_t = cp.tile([P, tpc], dt.float32)
            nc.sync.dma_start(onem_t[:], onem_d[:, :])

            w1_t = cp.tile([M, F1], dt.float32)
            nc.sync.dma_start(w1_t[:], w1_d[:, :])
            b1_t = cp.tile([1, F1], dt.float32)
            nc.sync.dma_start(b1_t[:], b1_d[:, :])
            w2f_t = cp.tile([F1, F2], dt.float32)
            nc.sync.dma_start(w2f_t[:], w2_d[:, :])
            w2_t = cp.tile([F1, F2], dt.bfloat16)
            nc.vector.tensor_copy(w2_t[:], w2f_t[:])
            b2_t = cp.tile([1, F2], dt.float32)
            nc.sync.dma_start(b2_t[:], b2_d[:, :])

            acc = cp.tile([P, F2], dt.float32)
            nc.vector.memset(acc[:], 1.0)

            # ---- phase 1: hs1 = (x @ W1) * dinv[node], node-major bf16 ----
            BG = 4  # node-blocks per psum tile
            with (
                tc.tile_pool(name="p1sb", bufs=4) as sp1,
                tc.tile_pool(name="p1ps", bufs=3, space="PSUM") as pp1,
            ):
                for g0 in range(0, TT, BG):
                    nb = min(BG, TT - g0)
                    xc = sp1.tile([M, BG * P], dt.float32, tag="xc")
                    nc.sync.dma_start(xc[:, : nb * P], xT_d[:, g0 * P : (g0 + nb) * P])
                    ps = pp1.tile([P, BG * F1], dt.float32, tag="ps0")
                    hsb = sp1.tile([P, BG * F1], dt.bfloat16, tag="hsb")
                    for j in range(nb):
                        b = g0 + j
                        nc.tensor.matmul(
                            ps[:, j * F1 : (j + 1) * F1],
                            xc[:, j * P : (j + 1) * P],
                            w1_t[:],
                            start=True, stop=True,
                        )
                        nc.scalar.activation(
                            hsb[:, j * F1 : (j + 1) * F1],
                            ps[:, j * F1 : (j + 1) * F1],
                            Act.Copy, scale=dinva_t[:, b : b + 1],
                        )
                    nc.sync.dma_start(
                        hs1_d.ap().rearrange("(t p) f -> p t f", p=P)[:, g0 : g0 + nb, :],
                        hsb[:, : nb * F1],
                    )

            tc.no_sync_barrier()

            # ---- phase 2: layer-1 conv + project to layer-2 table ----
            with (
                tc.tile_pool(name="p2sb", bufs=3) as sp2,
                tc.tile_pool(name="p2eq", bufs=8) as sp2e,
                tc.tile_pool(name="p2ps", bufs=2, space="PSUM") as pp2,
            ):
                for g in range(tpc // GT):
                    gt1 = sp2.tile([P, GT * cmax * F1], dt.bfloat16, tag="g1")
                    nc.gpsimd.indirect_dma_start(
                        out=gt1[:], out_offset=None,
                        in_=hs1_d[:, :],
                        in_offset=bass.IndirectOffsetOnAxis(
                            ap=idx_t[:, g * GT * cmax : (g + 1) * GT * cmax], axis=0),
                    )
                    for tl in range(GT):
                        t = g * GT + tl
                        ps1 = pp2.tile([P, F1], dt.float32, tag="ps1")
                        for c in range(cmax):
                            col = t * cmax + c
                            eq = sp2e.tile([P, P], dt.bfloat16, tag="eq")
                            nc.vector.tensor_scalar(
                                eq[:], iota_b[:], cl_t[:, col : col + 1], None,
                                Alu.is_equal)
                            nc.tensor.matmul(
                                ps1[:], eq[:],
                                gt1[:, (tl * cmax + c) * F1 : (tl * cmax + c + 1) * F1],
                                start=(c == 0), stop=False)
                        nc.tensor.matmul(
                            ps1[:], sqrow_t[:, t * P : (t + 1) * P], b1_t[:],
                            start=False, stop=True)
                        h1 = sp2.tile([P, F1], dt.bfloat16, tag="h1")
                        nc.scalar.activation(
                            h1[:], ps1[:], Act.Tanh, scale=dinvo_t[:, t : t + 1])
                        t2 = sp2.tile([P, F1], dt.bfloat16, tag="t2")
                        nc.scalar.activation(
                            t2[:], h1[:], Act.Copy, scale=dinvo_t[:, t : t + 1])
                        psT = pp2.tile([P, F1], dt.bfloat16, tag="psT")
                        nc.tensor.transpose(psT[:], t2[:], ident_b[:])
                        t2T = sp2.tile([P, F1], dt.bfloat16, tag="t2T")
                        nc.vector.tensor_copy(t2T[:], psT[:])
                        ps2 = pp2.tile([P, F2], dt.float32, tag="ps2")
                        nc.tensor.matmul(ps2[:], t2T[:], w2_t[:], start=True, stop=True)
                        hs2sb = sp2.tile([P, F2], dt.bfloat16, tag="hs2sb")
                        nc.scalar.activation(hs2sb[:], ps2[:], Act.Copy)
                        nc.sync.dma_start(hs2s_d[t * P : (t + 1) * P, :], hs2sb[:])

            tc.no_sync_barrier()

            # ---- phase 3: allgather layer-2 table ----
            if collectives:
                nc.gpsimd.collective_compute(
                    "AllGather", Alu.bypass, replica_groups=groups,
                    ins=[hs2s_d[:, :]], outs=[hs2_d[:, :]])
            else:
                nc.sync.dma_start(hs2_d[0 : tpc * P, :], hs2s_d[:, :])

            tc.no_sync_barrier()

            # ---- phase 4: layer-2 conv + masked product ----
            with (
                tc.tile_pool(name="p4sb", bufs=3) as sp4,
                tc.tile_pool(name="p4eq", bufs=8) as sp4e,
                tc.tile_pool(name="p4ps", bufs=2, space="PSUM") as pp4,
            ):
                for g in range(tpc // GT):
                    gt2 = sp4.tile([P, GT * cmax * F2], dt.bfloat16, tag="g2")
                    nc.gpsimd.indirect_dma_start(
                        out=gt2[:], out_offset=None,
                        in_=hs2_d[:, :],
                        in_offset=bass.IndirectOffsetOnAxis(
                            ap=idx_t[:, g * GT * cmax : (g + 1) * GT * cmax], axis=0),
                    )
                    for tl in range(GT):
                        t = g * GT + tl
                        ps1b = pp4.tile([P, F2], dt.float32, tag="ps1b")
                        for c in range(cmax):
                            col = t * cmax + c
                            eq = sp4e.tile([P, P], dt.bfloat16, tag="eq4")
                            nc.vector.tensor_scalar(
                                eq[:], iota_b[:], cl_t[:, col : col + 1], None,
                                Alu.is_equal)
                            nc.tensor.matmul(
                                ps1b[:], eq[:],
                                gt2[:, (tl * cmax + c) * F2 : (tl * cmax + c + 1) * F2],
                                start=(c == 0), stop=False)
                        nc.tensor.matmul(
                            ps1b[:], sqrow_t[:, t * P : (t + 1) * P], b2_t[:],
                            start=False, stop=True)
                        h2 = sp4.tile([P, F2], dt.float32, tag="h2")
                        nc.scalar.activation(
                            h2[:], ps1b[:], Act.Tanh, scale=dinvo_t[:, t : t + 1])
                        h2m = sp4.tile([P, F2], dt.float32, tag="h2m")
                        nc.vector.tensor_scalar(
                            h2m[:], h2[:], msk_t[:, t : t + 1], onem_t[:, t : t + 1],
                            Alu.mult, Alu.add)
                        nc.vector.tensor_mul(acc[:], acc[:], h2m[:])

            tc.no_sync_barrier()

            # ---- phase 5: partition product, cross-core product, MLP head ----
            with (
                tc.tile_pool(name="p5sb", bufs=1) as sp5,
                tc.tile_pool(name="p5ps", bufs=1, space="PSUM") as pp5,
            ):
                psa = pp5.tile([F2, P], dt.float32)
                nc.tensor.transpose(psa[:], acc[:], ident_f[:])
                accT = sp5.tile([F2, P], dt.float32)
                nc.vector.tensor_copy(accT[:], psa[:])
                w = P
                while w > 1:
                    w //= 2
                    nc.vector.tensor_mul(accT[:, :w], accT[:, :w], accT[:, w : 2 * w])
                nc.sync.dma_start(ag_in[:, :], accT[:, :1])
                if collectives:
                    nc.gpsimd.collective_compute(
                        "AllGather", Alu.bypass, replica_groups=groups,
                        ins=[ag_in[:, :]], outs=[ag_out[:, :]])
                else:
                    nc.sync.dma_start(ag_out[0:1, :], ag_in[:, :])
                all_t = sp5.tile([1, n_cores * F2], dt.float32)
                nc.sync.dma_start(all_t[:], ag_out[:, :])
                w = n_cores * F2
                while w > F2:
                    w //= 2
                    nc.vector.tensor_mul(all_t[:, :w], all_t[:, :w], all_t[:, w : 2 * w])
                nc.sync.dma_start(ptr_d[:, :], all_t[:, :F2])
                pT = sp5.tile([F2, 1], dt.float32)
                nc.sync.dma_start(pT[:], ptr_d[:, :])

                wd1_t = sp5.tile([F2, 128], dt.float32)
                nc.sync.dma_start(wd1_t[:], wd1_d[:, :])
                bd1_t = sp5.tile([128, 1], dt.float32)
                nc.sync.dma_start(bd1_t[:], bd1_d[:, :])
                ps1m = pp5.tile([128, 1], dt.float32)
                nc.tensor.matmul(ps1m[:], wd1_t[:], pT[:], start=True, stop=True)
                g1_t = sp5.tile([128, 1], dt.float32)
                nc.scalar.activation(g1_t[:], ps1m[:], Act.Tanh, bias=bd1_t[:, :1])

                wd2_t = sp5.tile([128, F2], dt.float32)
                nc.sync.dma_start(wd2_t[:], wd2_d[:, :])
                bd2_t = sp5.tile([F2, 1], dt.float32)
                nc.sync.dma_start(bd2_t[:], bd2_d[:, :])
                ps2m = pp5.tile([F2, 1], dt.float32)
                nc.tensor.matmul(ps2m[:], wd2_t[:], g1_t[:], start=True, stop=True)
                g2_t = sp5.tile([F2, 1], dt.float32)
                nc.scalar.activation(g2_t[:], ps2m[:], Act.Tanh, bias=bd2_t[:, :1])
                nc.sync.dma_start(g_d[0, :, None], g2_t[:])

                wo_t = sp5.tile([F2, 1], dt.float32)
                nc.sync.dma_start(wo_t[:], wo_d[:, :])
                pso = pp5.tile([1, 1], dt.float32)
                nc.tensor.matmul(pso[:], wo_t[:], g2_t[:], start=True, stop=True)
                bo_t = sp5.tile([1, 1], dt.float32)
                nc.sync.dma_start(bo_t[:], bo_d[:, :])
                o_t = sp5.tile([1, 1], dt.float32)
                nc.vector.tensor_add(o_t[:], pso[:], bo_t[:])
                nc.sync.dma_start(out_d[:, :], o_t[:])

    if split_waits:
        split_excess_waits(nc, limit=1)
    return nc


def make_in_maps(cfg, inputs, meta, shared, per_core):
    w1 = np.asarray(inputs["W1"], np.float32)
    b1 = np.asarray(inputs["b1"], np.float32)
    b2 = np.asarray(inputs["b2"], np.float32)
    common = dict(
        xT=shared["xT"], dega=shared["dega"],
        w1=w1, b1=b1[None, :], w2=np.asarray(inputs["W2"], np.float32), b2=b2[None, :],
        wd1=np.asarray(inputs["Wd1"], np.float32),
        bd1=np.asarray(inputs["bd1"], np.float32).reshape(-1, 1),
        wd2=np.asarray(inputs["Wd2"], np.float32),
        bd2=np.asarray(inputs["bd2"], np.float32).reshape(-1, 1),
        wo=np.asarray(inputs["Wo"], np.float32),
        bo=np.asarray(inputs["bo"], np.float32).reshape(1, 1),
    )
    return [dict(common, **pc) for pc in per_core]


def run(inputs, cfg=None, trace=False):
    cfg = cfg or FULL_CFG
    _register_profile_hook()
    meta, shared, per_core = prep(inputs["x"], inputs["edge_index"], cfg)
    global _LAST_CMAX
    _LAST_CMAX = meta["cmax"]
    nc = build_program(cfg, meta["cmax"])
    in_maps = make_in_maps(cfg, inputs, meta, shared, per_core)
    res = run_bass_kernel_spmd(
        nc, in_maps, list(range(cfg["n_cores"])), trace=trace)
    r0 = res.results[0]
    return (np.asarray(r0["out"], np.float32),
            np.asarray(r0["g"], np.float32)), res


def kernel(**inputs):
    (out, g), _ = run(inputs)
    return (out, g)
